# revision 5
# baseline (speedup 1.0000x reference)
"""Trainium2 Bass kernel for nn_LogMM: out = log(max(x @ matrix, tiny)).

Reference math: y = einsum('bsk,km->bsm', x, matrix); big = (y>0); small = 1-big;
out = log(max(y,eps))*big + log(max(y,eps))*small == log(max(y, eps)).
(y_big == y_small numerically, and big+small == 1 elementwise.)

Sharding: data-parallel over batch B=8, one batch slice per NeuronCore;
matrix replicated. Zero communication.

Per-core kernel: x_b [2048, 1024] @ matrix [1024, 1024] -> log -> out_b.
The contraction dim k must live on SBUF partitions for both matmul operands;
matrix is already [k, m], x tiles are transposed on-chip via PE transpose.
"""

import os
from contextlib import ExitStack

import numpy as np

import concourse.bass as bass
import concourse.bacc as bacc
import concourse.mybir as mybir
import concourse.tile as tile
from concourse.bass_utils import run_bass_kernel_spmd
from concourse.masks import make_identity

B, S, K, M = 8, 2048, 1024, 1024
P = 128
N_CORES = 8

# matmul input dtype: "f32" (exact, 4 cyc/row), "f32r" (fp32 bits, 1 cyc/row
# at N>=256), "bf16" (cast, 1 cyc/row)
MM_DT = os.environ.get("LOGMM_DT", "f32r")
N_TILE = 512


def _emit(ctx: ExitStack, tc: "tile.TileContext", out_ap, x_ap, mat_ap, mm_dt: str):
    nc = tc.nc
    S_TILES = S // P  # 16
    KO = K // P  # 8
    MO = M // N_TILE

    # dtype of the SBUF tiles fed to the accumulation matmuls
    if mm_dt == "bf16":
        mm_sb_dt = mybir.dt.bfloat16
    elif mm_dt == "f32r":
        mm_sb_dt = mybir.dt.float32r
    else:
        mm_sb_dt = mybir.dt.float32

    # x tiles are loaded [s, k] and transposed on PE; the transpose runs in
    # the load dtype (fp32 for f32/f32r, bf16 for bf16 via casting DMA).
    ld_dt = mybir.dt.bfloat16 if mm_dt == "bf16" else mybir.dt.float32

    const_pool = ctx.enter_context(tc.tile_pool(name="const", bufs=1))
    xin_pool = ctx.enter_context(tc.tile_pool(name="xin", bufs=3))
    xt_pool = ctx.enter_context(tc.tile_pool(name="xt", bufs=3))
    ob_pool = ctx.enter_context(tc.tile_pool(name="ob", bufs=4))
    pst_pool = ctx.enter_context(tc.tile_pool(name="pst", bufs=2, space="PSUM"))
    psm_pool = ctx.enter_context(tc.tile_pool(name="psm", bufs=4, space="PSUM"))

    ident = const_pool.tile([P, P], ld_dt)
    make_identity(nc, ident)

    # matrix -> SBUF [P(k_inner), KO(k_outer), M], natural k-on-partitions
    mat_sb = const_pool.tile([P, KO, M], mm_sb_dt)
    mat_src = mat_ap.rearrange("(ko p) m -> p ko m", p=P)
    if mm_sb_dt == mybir.dt.float32:
        nc.sync.dma_start(mat_sb[:], mat_src)
    elif mm_sb_dt == mybir.dt.bfloat16:
        nc.gpsimd.dma_start(mat_sb[:], mat_src)  # casting DMA
    else:  # f32r needs an explicit rounding op: DMA fp32 stage, DVE-round
        mat_stage = const_pool.tile([P, KO, M], mybir.dt.float32)
        nc.sync.dma_start(mat_stage[:], mat_src)
        for ko in range(KO):  # chunked so rounding overlaps other startup work
            nc.vector.tensor_copy(mat_sb[:, ko, :], mat_stage[:, ko, :])

    for st in range(S_TILES):
        s_sl = slice(st * P, (st + 1) * P)
        x_nat = xin_pool.tile([P, K], ld_dt)  # s on partitions, k free
        if ld_dt == mybir.dt.float32:
            nc.sync.dma_start(x_nat[:], x_ap[s_sl, :])
        else:
            nc.gpsimd.dma_start(x_nat[:], x_ap[s_sl, :])

        # transpose x tile: [s, k] -> [k, s] via PE, 128x128 blocks;
        # the PSUM->SBUF copy also rounds to the matmul dtype.
        xT = xt_pool.tile([P, KO, P], mm_sb_dt)
        for ko in range(KO):
            ps = pst_pool.tile([P, P], ld_dt)
            nc.tensor.transpose(
                ps[:], x_nat[:, ko * P : (ko + 1) * P], ident[:]
            )
            nc.vector.tensor_copy(xT[:, ko, :], ps[:])

        for mo in range(MO):
            m_sl = slice(mo * N_TILE, (mo + 1) * N_TILE)
            pm = psm_pool.tile([P, N_TILE], mybir.dt.float32)
            for ko in range(KO):
                nc.tensor.matmul(
                    pm[:],
                    xT[:, ko, :],
                    mat_sb[:, ko, m_sl],
                    start=(ko == 0),
                    stop=(ko == KO - 1),
                )
            ob = ob_pool.tile([P, N_TILE], mybir.dt.float32)
            nc.scalar.activation(ob[:], pm[:], mybir.ActivationFunctionType.Ln)
            nc.sync.dma_start(out_ap[s_sl, m_sl], ob[:])


def _build_nc(mm_dt: str):
    nc = bacc.Bacc("TRN2", target_bir_lowering=False, debug=False)
    x = nc.dram_tensor("x", [S, K], mybir.dt.float32, kind="ExternalInput").ap()
    mat = nc.dram_tensor("matrix", [K, M], mybir.dt.float32, kind="ExternalInput").ap()
    out = nc.dram_tensor("out", [S, M], mybir.dt.float32, kind="ExternalOutput").ap()
    with tile.TileContext(nc) as tc:
        with ExitStack() as ctx:
            _emit(ctx, tc, out, x, mat, mm_dt)
    nc.compile()
    return nc


_nc_cache: dict = {}


def _get_nc(mm_dt: str):
    if mm_dt not in _nc_cache:
        _nc_cache[mm_dt] = _build_nc(mm_dt)
    return _nc_cache[mm_dt]


def kernel(x: np.ndarray, matrix: np.ndarray, _trace: bool = False):
    assert x.shape == (B, S, K) and matrix.shape == (K, M)
    nc = _get_nc(MM_DT)
    x = np.ascontiguousarray(x, dtype=np.float32)
    matrix = np.ascontiguousarray(matrix, dtype=np.float32)
    in_maps = [{"x": x[b], "matrix": matrix} for b in range(N_CORES)]
    res = run_bass_kernel_spmd(nc, in_maps, core_ids=list(range(N_CORES)), trace=_trace)
    out = np.stack([r["out"] for r in res.results], axis=0)
    if _trace:
        kernel.last_results = res  # stash for profiling inspection
    return out
